# revision 46
# baseline (speedup 1.0000x reference)
"""TRN2 Bass kernel for nn_Block_82325933129820.

3x AFT blocks + 1 transformer (TEA) block, B=4 T=1024 E=1024 QKV=2048 H=16.

Sharding: 8 cores = 4 batch-pairs. Within a pair (even core, odd core):
  - AFT layers: token-split (even: tokens 0-511, odd: 512-1023), feature-major
    activations (channels on partitions, tokens on free dim). The cumsum runs
    as per-chunk tensor_tensor_scan along the free dim; cross-core carries
    travel via pair AllGathers and enter as the scan's `initial` value, gated
    to zero on even cores (with the denominator's +1e-6 folded in).
  - TEA: head-split (even: heads 0-7, odd: 8-15) over the full 1024 tokens.
    x3 is pair-AllGathered in bf16; attention is computed in S^T layout; the
    swiglu partial contraction is pair-ReduceScattered in bf16.

Precision/perf strategy: all GEMM weights are bf16 (half the HBM traffic);
PSUM accumulation is fp32. AFT intermediate activations (q/k/w/wv/yf) are
bf16 which enables DVE 2x modes; the cumsum scan recurrence is fp32
internally regardless. TEA attention internals stay fp32 (f32r matmuls at
full PE rate for N>=512). All reciprocals run on the scalar engine as
Exp(-Ln(x)); rsqrt(x) = Exp(-0.5*Ln(x)); sigmoid/silu via Exp with the
reciprocal folded into existing products; "+1"/"+eps" constants folded into
activation bias / scan initials. The AFT swiglu's first 8 output tiles
accumulate c-interleaved with the cumsum pipeline so the tensor engine
stays busy through the vector-heavy phase.
"""
import os
import sys
import numpy as np
import ml_dtypes

for _p in ('/opt/trn_rl_repo',):
    if _p not in sys.path:
        sys.path.insert(0, _p)

import concourse.bass as bass
import concourse.mybir as mybir
import concourse.tile as tile
from concourse import bacc
from concourse.bass_utils import run_bass_kernel_spmd

P = 128
TL = 512          # AFT tokens per core
E = 1024
QKV = 2048
T = 1024
DH = 128
NCORES = 8
NE = E // P       # 8
NC = QKV // P     # 16
EPS = float(np.finfo(np.float32).eps)
f32 = mybir.dt.float32
f32r = mybir.dt.float32r
bf16 = mybir.dt.bfloat16
AF = mybir.ActivationFunctionType
ALU = mybir.AluOpType
PAIRS = [[0, 1], [2, 3], [4, 5], [6, 7]]
BF = ml_dtypes.bfloat16


def _rsqrt(nc, pool, src_ps, scale, bias_ap, tag, ln_bufs=None):
    """rsqrt(src*scale + bias) = Exp(-0.5*Ln(.)). src_ps is PSUM (P, n)."""
    n = src_ps.shape[-1]
    tmp = pool.tile([P, n], f32, tag="lntmp", bufs=ln_bufs)
    nc.scalar.activation(tmp[:], src_ps[:], AF.Ln, scale=scale, bias=bias_ap)
    out = pool.tile([P, n], bf16, tag=tag)
    nc.scalar.activation(out[:], tmp[:], AF.Exp, scale=-0.5)
    return out


def _wgroup(nc, pool, wdram, m0, G, K=E, tag="wkg", bufs=None, name="wt",
            splits=2):
    """(P, G, K//P, P) bf16 weight group from host-packed (P, Mtiles*K) DRAM.

    Row p of the DRAM tensor holds tile-m-major data, so a G-tile load is
    one G*K*2-byte contiguous chunk per partition (fat DMA descriptors).
    The load is issued as `splits` partition-sliced dma_starts so several
    hardware queues stream concurrently (per-stream DMA bw is the limiter).
    """
    wt = pool.tile([P, G, K // P, P], bf16, tag=tag, bufs=bufs, name=name)
    step = P // splits
    for s in range(splits):
        lo, hi = s * step, (s + 1) * step
        nc.sync.dma_start(wt[lo:hi], wdram.ap()[lo:hi, m0 * K:(m0 + G) * K]
                          .rearrange("p (b a n) -> p b a n", b=G, n=P))
    return wt


def _build_aft_layer(tc, const, x_tiles, xp, wqkvT, wswiU, wswiG,
                     woutT, ag_ins, ag_outs, x3_bf=None):
    """One AFT layer, fully SBUF-resident activations.

    x_tiles: list of 8 (P, TL) f32 SBUF tiles (residual stream).
    Returns the new list of 8 x tiles (allocated from xp).
    If x3_bf is given (layer 3), also writes the bf16 output to that DRAM AP.
    """
    nc = tc.nc
    ones_b = const["ones_b"]
    gate_col = const["gate"]

    with (
        tc.tile_pool(name="a_sc", bufs=2) as scp,
        tc.tile_pool(name="a_k", bufs=NC) as kp,
        tc.tile_pool(name="a_q", bufs=NC) as qp,
        tc.tile_pool(name="a_ww", bufs=NC) as wwp,
        tc.tile_pool(name="a_yf", bufs=NC) as yfp,
        tc.tile_pool(name="a_cc", bufs=8) as ccp,
        tc.tile_pool(name="a_xn", bufs=NE) as xnp,
    ):
        yf_t = [None] * NC
        w_t = [None] * NC
        wv_t = [None] * NC
        with (
            tc.tile_pool(name="a_w8", bufs=4) as wp,
            tc.tile_pool(name="a_ld", bufs=4) as sbp,
            tc.tile_pool(name="a_ps", bufs=4, space="PSUM") as ps,
            tc.tile_pool(name="a_ps2", bufs=1, space="PSUM") as ps2,
        ):
            # ---- rms(x) ----
            xsq = []
            for e in range(NE):
                t = sbp.tile([P, TL], bf16, tag="sq", bufs=NE)
                nc.gpsimd.tensor_tensor(t[:], x_tiles[e][:], x_tiles[e][:],
                                        ALU.mult)
                xsq.append(t)
            sumsq = ps2.tile([P, TL], f32, tag="xsumsq")
            for e in range(NE):
                nc.tensor.matmul(sumsq[:], ones_b[:], xsq[e][:],
                                 start=(e == 0), stop=(e == NE - 1))
            xscale = _rsqrt(nc, scp, sumsq, 1.0 / E, const["epsb"][:],
                            "scale")
            xn = []
            for e in range(NE):
                t = xnp.tile([P, TL], bf16, tag="xn")
                nc.gpsimd.tensor_tensor(t[:], x_tiles[e][:], xscale[:],
                                        ALU.mult)
                xn.append(t)

            def qkv_group(mt0, gblk):
                """Load G=4 qkv weight tiles, return the group tile."""
                return _wgroup(nc, wp, wqkvT, mt0 + 4 * gblk, 4, tag="wk4",
                               splits=1)

            def qkv_acc(wt, b):
                acc = ps.tile([P, TL], f32, tag="mm", name="acc")
                for e in range(NE):
                    nc.tensor.matmul(acc[:], wt[:, b, e, :], xn[e][:],
                                     start=(e == 0), stop=(e == NE - 1))
                return acc

            # ---- k tiles (SBUF-resident bf16); k weight tiles are m 16..31
            k_sb = [None] * NC
            ksq = [None] * NC
            for gblk in range(4):
                wt = qkv_group(16, gblk)
                for b in range(4):
                    c = 4 * gblk + b
                    acc = qkv_acc(wt, b)
                    kt = kp.tile([P, TL], bf16, tag="k")
                    nc.scalar.copy(kt[:], acc[:])
                    k_sb[c] = kt
                    sq = sbp.tile([P, TL], bf16, tag="sq", bufs=NE)
                    nc.gpsimd.tensor_tensor(sq[:], kt[:], kt[:], ALU.mult)
                    ksq[c] = sq
            ksumsq = ps2.tile([P, TL], f32, tag="ksumsq")
            for c in range(NC):
                nc.tensor.matmul(ksumsq[:], ones_b[:], ksq[c][:],
                                 start=(c == 0), stop=(c == NC - 1))
            kscale = _rsqrt(nc, scp, ksumsq, 1.0 / QKV, const["epsb"][:],
                            "scale")

            # ---- v matmuls + w/wv + carries (2 groups of 8); v is m 32..47
            for g in range(2):
                for gblk in range(2 * g, 2 * g + 2):
                    wt = qkv_group(32, gblk)
                    for b in range(4):
                        c = 4 * gblk + b
                        kn = sbp.tile([P, TL], bf16, tag="kn", bufs=3)
                        nc.gpsimd.tensor_tensor(kn[:], k_sb[c][:],
                                                kscale[:], ALU.mult)
                        w = wwp.tile([P, TL], bf16, tag="w")
                        cw_col = ccp.tile([P, 1], f32, tag="cwc")
                        nc.scalar.activation(w[:], kn[:], AF.Exp,
                                             accum_out=cw_col[:])
                        acc = qkv_acc(wt, b)
                        wv = wwp.tile([P, TL], bf16, tag="wv")
                        cwv_col = ccp.tile([P, 1], f32, tag="cwvc")
                        nc.vector.scalar_tensor_tensor(
                            wv[:], acc[:], 0.0, w[:], ALU.bypass, ALU.mult,
                            accum_out=cwv_col[:])
                        j = c - 8 * g
                        nc.sync.dma_start(
                            ag_ins[g].opt()[:, j * P:(j + 1) * P]
                            .rearrange("o (p q) -> p (o q)", p=P),
                            cwv_col[:])
                        nc.sync.dma_start(
                            ag_ins[g].opt()[:, 1024 + j * P:1024 + (j + 1) * P]
                            .rearrange("o (p q) -> p (o q)", p=P),
                            cw_col[:])
                        w_t[c] = w
                        wv_t[c] = wv
                nc.gpsimd.collective_compute(
                    "AllGather", ALU.bypass, replica_groups=PAIRS,
                    ins=[ag_ins[g].opt()], outs=[ag_outs[g].opt()])

            # ---- q tiles (SBUF-resident bf16); q is m 0..15 ----
            q_sb = [None] * NC
            qsq = [None] * NC
            for gblk in range(4):
                wt = qkv_group(0, gblk)
                for b in range(4):
                    c = 4 * gblk + b
                    acc = qkv_acc(wt, b)
                    qt = qp.tile([P, TL], bf16, tag="q")
                    nc.scalar.copy(qt[:], acc[:])
                    q_sb[c] = qt
                    sq = sbp.tile([P, TL], bf16, tag="sq", bufs=NE)
                    nc.gpsimd.tensor_tensor(sq[:], qt[:], qt[:], ALU.mult)
                    qsq[c] = sq
            qsumsq = ps2.tile([P, TL], f32, tag="qsumsq")
            for c in range(NC):
                nc.tensor.matmul(qsumsq[:], ones_b[:], qsq[c][:],
                                 start=(c == 0), stop=(c == NC - 1))
            qscale = _rsqrt(nc, scp, qsumsq, 1.0 / QKV, const["epsb"][:],
                            "scale")

        # ---- phase B (scans etc.) interleaved with swiglu pass 1 ----
        with (
            tc.tile_pool(name="a_sw", bufs=2) as swp,
            tc.tile_pool(name="a_pb", bufs=2) as pbp,
            tc.tile_pool(name="a_u", bufs=NE) as up,
            tc.tile_pool(name="a_mt", bufs=NE) as mtp,
            tc.tile_pool(name="a_pss", bufs=8, space="PSUM") as pss,
        ):
            sacc = [None] * NE
            for g in range(2):
                cwv_raw = ccp.tile([P, 8], f32, tag="cwvr")
                nc.sync.dma_start(
                    cwv_raw[:], ag_outs[g].opt()[0:1, 0:1024]
                    .rearrange("o (c p) -> p (o c)", p=P))
                cw_raw = ccp.tile([P, 8], f32, tag="cwr")
                nc.sync.dma_start(
                    cw_raw[:], ag_outs[g].opt()[0:1, 1024:2048]
                    .rearrange("o (c p) -> p (o c)", p=P))
                cwv_g = ccp.tile([P, 8], f32, tag="cwvg")
                nc.vector.tensor_scalar(cwv_g[:], cwv_raw[:],
                                        gate_col[:], None, ALU.mult)
                # denominator carry gets the +1e-6 folded in
                cw_g = ccp.tile([P, 8], f32, tag="cwg")
                nc.vector.tensor_scalar(cw_g[:], cw_raw[:],
                                        gate_col[:], 1e-6,
                                        ALU.mult, ALU.add)
                for c in range(8 * g, 8 * g + 8):
                    j = c - 8 * g
                    sw = pbp.tile([P, TL], bf16, tag="sw")
                    nc.vector.tensor_tensor_scan(
                        sw[:], wv_t[c][:], wv_t[c][:], cwv_g[:, j:j + 1],
                        ALU.add, ALU.bypass)
                    sw2 = pbp.tile([P, TL], bf16, tag="sw2")
                    nc.vector.tensor_tensor_scan(
                        sw2[:], w_t[c][:], w_t[c][:], cw_g[:, j:j + 1],
                        ALU.add, ALU.bypass)
                    qn = pbp.tile([P, TL], bf16, tag="qn")
                    nc.gpsimd.tensor_tensor(qn[:], q_sb[c][:], qscale[:],
                                            ALU.mult)
                    et = pbp.tile([P, TL], bf16, tag="et")
                    nc.scalar.activation(et[:], qn[:], AF.Exp, scale=-1.0)
                    # dd = (et + 1) * sw2   (sw2 already carries the +1e-6)
                    dd = pbp.tile([P, TL], bf16, tag="dd")
                    nc.vector.scalar_tensor_tensor(
                        dd[:], et[:], 1.0, sw2[:], ALU.add, ALU.mult)
                    lnd = pbp.tile([P, TL], bf16, tag="lnd")
                    nc.scalar.activation(lnd[:], dd[:], AF.Ln)
                    rr = pbp.tile([P, TL], bf16, tag="rr")
                    nc.scalar.activation(rr[:], lnd[:], AF.Exp, scale=-1.0)
                    yf = yfp.tile([P, TL], bf16, tag="yf")
                    nc.vector.tensor_tensor(yf[:], sw[:], rr[:], ALU.mult)
                    yf_t[c] = yf
                    # swiglu pass 1 (u half, m=0..7), c-interleaved
                    if c % 2 == 0:
                        w1 = _wgroup(nc, swp, wswiU, c, 2, tag="w1",
                                     name="w1", splits=1)
                    for m in range(NE):
                        if c == 0:
                            sacc[m] = pss.tile([P, TL], f32, tag="sacc", name="sacc")
                        nc.tensor.matmul(sacc[m][:], w1[:, c % 2, m, :],
                                         yf[:],
                                         start=(c == 0), stop=(c == NC - 1))

            # drain u, then swiglu pass 2 (g half, m=8..15), c-outer
            u_sb = [None] * NE
            for m in range(NE):
                ut = up.tile([P, TL], bf16, tag="u")
                nc.scalar.copy(ut[:], sacc[m][:])
                u_sb[m] = ut
            sacc2 = [None] * NE
            for c in range(NC):
                if c % 2 == 0:
                    w2 = _wgroup(nc, swp, wswiG, c, 2, tag="w2", name="w2",
                                 splits=1)
                for m in range(NE):
                    if c == 0:
                        sacc2[m] = pss.tile([P, TL], f32, tag="sacc", name="sacc2")
                    nc.tensor.matmul(sacc2[m][:], w2[:, c % 2, m, :],
                                     yf_t[c][:],
                                     start=(c == 0), stop=(c == NC - 1))
            # silu: m = u * g / (1 + exp(-g))
            m_t = [None] * NE
            for m in range(NE):
                eg = pbp.tile([P, TL], bf16, tag="eg")
                nc.scalar.activation(eg[:], sacc2[m][:], AF.Exp, scale=-1.0)
                lnd = pbp.tile([P, TL], bf16, tag="lnd")
                nc.scalar.activation(lnd[:], eg[:], AF.Ln,
                                     bias=const["oneb"][:])
                rr = pbp.tile([P, TL], bf16, tag="rr")
                nc.scalar.activation(rr[:], lnd[:], AF.Exp, scale=-1.0)
                pug = pbp.tile([P, TL], bf16, tag="pug")
                nc.vector.tensor_tensor(pug[:], u_sb[m][:], sacc2[m][:],
                                        ALU.mult)
                mt = mtp.tile([P, TL], bf16, tag="mt")
                nc.gpsimd.tensor_tensor(mt[:], pug[:], rr[:], ALU.mult)
                m_t[m] = mt

            # ---- out-proj + residual (SBUF resident) ----
            new_x = []
            with tc.tile_pool(name="a_w8b", bufs=2) as wpb:
                for mo in range(NE):
                    if mo % 4 == 0:
                        wo = _wgroup(nc, wpb, woutT, mo, 4, tag="wo",
                                     name="wo")
                    acc = pss.tile([P, TL], f32, tag="sacc", name="oacc")
                    for c in range(NE):
                        nc.tensor.matmul(acc[:], wo[:, mo % 4, c, :],
                                         m_t[c][:],
                                         start=(c == 0), stop=(c == NE - 1))
                    xo = xp.tile([P, TL], f32, tag="x", bufs=10)
                    nc.vector.tensor_tensor(xo[:], acc[:], x_tiles[mo][:],
                                            ALU.add)
                    new_x.append(xo)
                    if x3_bf is not None:
                        xob = pbp.tile([P, TL], bf16, tag="xob")
                        nc.scalar.copy(xob[:], xo[:])
                        nc.sync.dma_start(
                            x3_bf[mo * P:(mo + 1) * P, :], xob[:])
    return new_x


def _build_tea(tc, const, x_tiles, wqk4c, wv4c, wswiT4c, woutT4,
               agx_out_h, rs_in_q, rs_out_q, outT):
    nc = tc.nc
    ones_r = const["ones_r"]
    cc_t, ss_t, cm_t = const["cc"], const["ss"], const["cmask"]
    HL = 8

    with (
        tc.tile_pool(name="t_yt", bufs=2 * HL) as ytp,
        tc.tile_pool(name="t_sc", bufs=2) as scp,
        tc.tile_pool(name="t_ps", bufs=2, space="PSUM") as ps,
        tc.tile_pool(name="t_ps2", bufs=2, space="PSUM") as ps2,
        tc.tile_pool(name="t_xn", bufs=2 * NE) as xnp,
        tc.tile_pool(name="t_v", bufs=16) as vp,
    ):
        with tc.tile_pool(name="t_t", bufs=3) as sbp:
            # ---- rms(x3) (x3 arrives bf16 via the pair AllGather) ----
            xn = [[None] * NE for _ in range(2)]
            for tch in range(2):
                def _x3_ap(tch, e):
                    half, er = e // 4, e % 4
                    return agx_out_h[half].opt()[
                        tch * (E // 2) + er * P:tch * (E // 2) + (er + 1) * P, :]

                xt3s = []
                for e in range(NE):
                    xt3 = sbp.tile([P, TL], bf16, tag="xt3", bufs=NE)
                    nc.sync.dma_start(xt3[:], _x3_ap(tch, e))
                    xt3s.append(xt3)
                sumsq = ps2.tile([P, TL], f32, tag="sumsq")
                for e in range(NE):
                    xsq = sbp.tile([P, TL], bf16, tag="sq")
                    nc.gpsimd.tensor_tensor(xsq[:], xt3s[e][:], xt3s[e][:],
                                            ALU.mult)
                    nc.tensor.matmul(sumsq[:], const["ones_b"][:], xsq[:],
                                     start=(e == 0), stop=(e == NE - 1))
                xscale = _rsqrt(nc, scp, sumsq, 1.0 / E, const["epsb"][:],
                                "xscale", ln_bufs=2)
                for e in range(NE):
                    t = xnp.tile([P, TL], bf16, tag="xn")
                    nc.vector.tensor_tensor(t[:], xt3s[e][:], xscale[:],
                                            ALU.mult)
                    xn[tch][e] = t

            # ---- V (token-major) ----
            V = [[None] * 2 for _ in range(8)]
            with tc.tile_pool(name="t_vw", bufs=2) as vwp:
                for vb in range(2):
                    vw = vwp.tile([P, NE, TL], bf16, tag="vw")
                    nc.sync.dma_start(
                        vw[:],
                        wv4c.ap()[vb * P:(vb + 1) * P, :]
                        .rearrange("p (a n) -> p a n", n=TL))
                    for ttile in range(8):
                        tch, toff = ttile // 4, (ttile % 4) * P
                        acc = ps.tile([P, TL], f32, tag="mm")
                        for e in range(NE):
                            nc.tensor.matmul(
                                acc[:], xn[tch][e][:, toff:toff + P],
                                vw[:, e, :],
                                start=(e == 0), stop=(e == NE - 1))
                        vt = vp.tile([P, TL], bf16, tag="V")
                        nc.scalar.copy(vt[:], acc[:])
                        V[ttile][vb] = vt

        # ---- per-head rope/rms + attention ----
        yT = [[None] * 2 for _ in range(HL)]
        with (
            tc.tile_pool(name="t_qk", bufs=8) as qkp,
            tc.tile_pool(name="t_es", bufs=8) as esp,
            tc.tile_pool(name="t_w8", bufs=3) as wp,
            tc.tile_pool(name="t_at", bufs=2) as sba,
            tc.tile_pool(name="t_psa", bufs=2, space="PSUM") as psa,
            tc.tile_pool(name="t_psd", bufs=1, space="PSUM") as psd,
        ):
            sel4 = const["sel4"]

            def qk_phase(h):
                """QK matmuls + rope + rms-scale for head h; returns
                (qn_h, kn_h) f32r SBUF tiles."""
                qn_h = [None] * 2
                kn_h = [None] * 2
                sites = []
                coll = scp.tile([4, TL], f32, tag="coll", bufs=2,
                                name="coll")
                wqk = _wgroup(nc, wp, wqk4c, 2 * h, 2, tag="wqk", name="wqk")
                for wi, out_list in enumerate((qn_h, kn_h)):
                    for tch in range(2):
                        acc = ps.tile([P, TL], f32, tag="mm", name="acc")
                        for e in range(NE):
                            nc.tensor.matmul(acc[:], wqk[:, wi, e, :],
                                             xn[tch][e][:],
                                             start=(e == 0),
                                             stop=(e == NE - 1))
                        zsq = sba.tile([P, TL], f32r, tag="sq", name="zsq")
                        nc.scalar.activation(zsq[:], acc[:], AF.Square)
                        sq_ps = ps2.tile([1, TL], f32, tag="sumsq",
                                         name="sq_ps")
                        nc.tensor.matmul(sq_ps[:], ones_r[:, 0:1], zsq[:],
                                         start=True, stop=True)
                        r = 2 * wi + tch
                        srow = scp.tile([1, TL], f32, tag="srow", bufs=3,
                                        name="srow")
                        nc.scalar.copy(srow[:], sq_ps[:])
                        nc.sync.dma_start(coll[r:r + 1, :], srow[:])
                        tsl = slice(tch * TL, (tch + 1) * TL)
                        tmp1 = sba.tile([P, TL], f32, tag="tmp1",
                                        name="tmp1")
                        nc.vector.tensor_tensor(tmp1[:], acc[:],
                                                cc_t[:, tsl], ALU.mult)
                        cross = sba.tile([P, TL], f32, tag="cross",
                                         name="cross")
                        nc.vector.tensor_tensor(cross[:64, :], acc[64:, :],
                                                ss_t[:64, tsl], ALU.mult)
                        nc.vector.tensor_tensor(cross[64:, :], acc[:64, :],
                                                ss_t[64:, tsl], ALU.mult)
                        zrope = sba.tile([P, TL], f32, tag="zrope",
                                         bufs=6, name="zrope")
                        nc.gpsimd.tensor_tensor(zrope[:], tmp1[:], cross[:],
                                                ALU.add)
                        sites.append((r, zrope, out_list, tch))
                # one Ln + one Exp for all 4 sites of this head.
                lnc = scp.tile([4, TL], f32, tag="lnc", bufs=2, name="lnc")
                nc.scalar.activation(lnc[:], coll[:], AF.Ln,
                                     bias=const["epsbdh"][0:4, :])
                esc = scp.tile([4, TL], f32r, tag="esc", bufs=2, name="esc")
                nc.scalar.activation(esc[:], lnc[:], AF.Exp, scale=-0.5,
                                     bias=const["klnb"][:])
                for r, zrope, out_list, tch in sites:
                    sc_ps = ps.tile([P, TL], f32, tag="mm", name="sc_ps")
                    nc.tensor.matmul(sc_ps[:], sel4[:, r * P:(r + 1) * P],
                                     esc[:], start=True, stop=True)
                    zn = qkp.tile([P, TL], f32r, tag="zn", name="zn")
                    nc.vector.tensor_tensor(zn[:], zrope[:], sc_ps[:],
                                            ALU.mult)
                    out_list[tch] = zn
                return qn_h, kn_h

            def attn_phase(h, qn_h, kn_h):
                for qc in range(2):
                    denom = psd.tile([P, TL], f32, tag="denom")
                    ytil = psd.tile([P, TL], f32, tag="ytil")
                    nkt = 4 * (qc + 1)
                    for kt in range(nkt):
                        tch_k, koff = kt // 4, (kt % 4) * P
                        sT = psa.tile([P, TL], f32, tag="sT")
                        nc.tensor.matmul(sT[:],
                                         kn_h[tch_k][:, koff:koff + P],
                                         qn_h[qc][:], start=True, stop=True)
                        es = esp.tile([P, TL], bf16, tag="es")
                        j = kt - 4 * qc
                        if j >= 0:
                            sm = sba.tile([P, TL], f32, tag="sm")
                            nc.vector.tensor_tensor(
                                sm[:], sT[:], cm_t[:, j * TL:(j + 1) * TL],
                                ALU.add)
                            nc.scalar.activation(es[:], sm[:], AF.Exp)
                        else:
                            nc.scalar.activation(es[:], sT[:], AF.Exp)
                        nc.tensor.matmul(denom[:], const["ones_b"][:], es[:],
                                         start=(kt == 0),
                                         stop=(kt == nkt - 1))
                        nc.tensor.matmul(
                            ytil[:],
                            V[kt][h // 4][:, (h % 4) * P:(h % 4 + 1) * P],
                            es[:], start=(kt == 0), stop=(kt == nkt - 1))
                    lnr = sba.tile([P, TL], f32, tag="lnr")
                    nc.scalar.activation(lnr[:], denom[:], AF.Ln)
                    rr = sba.tile([P, TL], f32, tag="arr")
                    nc.scalar.activation(rr[:], lnr[:], AF.Exp, scale=-1.0)
                    yt = ytp.tile([P, TL], bf16, tag="yT")
                    nc.vector.tensor_tensor(yt[:], ytil[:], rr[:], ALU.mult)
                    yT[h][qc] = yt

            # software-pipeline: head h's norm-collection latency hides
            # under head h-1's attention matmuls
            pend = None
            for h in range(HL):
                qk = qk_phase(h)
                if pend is not None:
                    attn_phase(pend[0], *pend[1])
                pend = (h, qk)
            attn_phase(pend[0], *pend[1])

        # ---- partial swiglu, 4 ReduceScatter chunks pipelined ----
        # chunk j covers m-tiles {2j, 2j+1, 8+2j, 8+2j+1} (u-pair + g-pair);
        # wswiT4c is host-packed in exactly this consumption order.
        with (
            tc.tile_pool(name="t_w8s", bufs=2) as wps,
            tc.tile_pool(name="t_pug", bufs=4) as pugp,
        ):
            for j in range(4):
                wt = _wgroup(nc, wps, wswiT4c, 4 * j, 4, tag="ws",
                             name="ws")
                for s in range(4):
                    for tch in range(2):
                        acc = ps.tile([P, TL], f32, tag="mm")
                        for kk in range(HL):
                            nc.tensor.matmul(acc[:], wt[:, s, kk, :],
                                             yT[kk][tch][:],
                                             start=(kk == 0),
                                             stop=(kk == HL - 1))
                        pug = pugp.tile([P, TL], bf16, tag="pug")
                        nc.scalar.copy(pug[:], acc[:])
                        nc.sync.dma_start(
                            rs_in_q[j].opt()[tch * 4 * P + s * P:
                                             tch * 4 * P + (s + 1) * P, :],
                            pug[:])
                nc.gpsimd.collective_compute(
                    "ReduceScatter", ALU.add, replica_groups=PAIRS,
                    ins=[rs_in_q[j].opt()], outs=[rs_out_q[j].opt()])

        # ---- silu + out-proj (c-outer, overlaps RS chunks) + residual ----
        with (
            tc.tile_pool(name="t_mt", bufs=NE) as mtp,
            tc.tile_pool(name="t_w8o", bufs=1) as wpo,
            tc.tile_pool(name="t_t4", bufs=2) as sb4,
            tc.tile_pool(name="t_pso", bufs=4, space="PSUM") as pso,
        ):
            woc = wpo.tile([P, NE, NE, P], bf16, tag="woc", name="woc")
            nc.sync.dma_start(woc[:], woutT4.ap()
                              .rearrange("p (c b n) -> p c b n", c=NE, n=P))
            m_t = [None] * NE
            for j in range(4):
                for i in range(2):
                    c = 2 * j + i
                    ut = sb4.tile([P, TL], bf16, tag="u4")
                    nc.sync.dma_start(
                        ut[:], rs_out_q[j].opt()[i * P:(i + 1) * P, :])
                    gt = sb4.tile([P, TL], bf16, tag="g4")
                    nc.sync.dma_start(
                        gt[:],
                        rs_out_q[j].opt()[(2 + i) * P:(3 + i) * P, :])
                    eg = sb4.tile([P, TL], bf16, tag="eg4")
                    nc.scalar.activation(eg[:], gt[:], AF.Exp, scale=-1.0)
                    lnd = sb4.tile([P, TL], bf16, tag="lnd4")
                    nc.scalar.activation(lnd[:], eg[:], AF.Ln,
                                         bias=const["oneb"][:])
                    rr = sb4.tile([P, TL], bf16, tag="rr4")
                    nc.scalar.activation(rr[:], lnd[:], AF.Exp, scale=-1.0)
                    pug = sb4.tile([P, TL], bf16, tag="pug4")
                    nc.gpsimd.tensor_tensor(pug[:], ut[:], gt[:], ALU.mult)
                    mt = mtp.tile([P, TL], bf16, tag="mt4")
                    nc.vector.tensor_tensor(mt[:], pug[:], rr[:], ALU.mult)
                    m_t[c] = mt
            oacc = [None] * NE
            for half in range(2):
                for c in range(NE):
                    for mo in range(4 * half, 4 * half + 4):
                        if c == 0:
                            oacc[mo] = pso.tile([P, TL], f32, tag="oacc",
                                                name="oacc")
                        nc.tensor.matmul(oacc[mo][:], woc[:, c, mo, :],
                                         m_t[c][:],
                                         start=(c == 0), stop=(c == NE - 1))
                for mo in range(4 * half, 4 * half + 4):
                    xo = sb4.tile([P, TL], f32, tag="xo4")
                    nc.vector.tensor_tensor(xo[:], oacc[mo][:],
                                            x_tiles[mo][:], ALU.add)
                    nc.sync.dma_start(outT.ap()[mo * P:(mo + 1) * P, :],
                                      xo[:])


class _Bacc(bacc.Bacc):
    """Bacc with the combined ln+exp activation table given priority.

    The act-table insertion pass assigns each activation the first table
    in the list that contains its function; the default act_info order
    makes Exp resolve to `exp_and_others` and Ln to `natural_log`, so a
    kernel that alternates Exp/Ln (reciprocals, rsqrt) reloads the table
    on nearly every call (~1.3us each). Putting
    `natural_log_exp_and_others` first lets Exp/Ln/Square/Copy all share
    one resident table.
    """

    def insert_act_table_loads(self):
        import bass_rust as _bass_rust
        from concourse.hw_specs import get_activation_tables
        has_activation = any(
            isinstance(i, mybir.InstActivation)
            for b in self.main_func.blocks
            for i in b.instructions
        )
        if not has_activation:
            return
        steer = {AF.Exp, AF.Ln, AF.Square, AF.Copy}
        tables = [
            (nm, set(fns) if nm == 'natural_log_exp_and_others'
             else set(fns) - steer)
            for nm, fns in get_activation_tables(self.m.arch).items()
        ]
        _bass_rust.insert_act_table_loads(self, tables)


def build_program():
    nc = _Bacc("TRN2", target_bir_lowering=False, debug=False,
               num_devices=NCORES)

    din = {}

    def inp(name, shape, dt):
        din[name] = nc.dram_tensor(name, list(shape), dt,
                                   kind="ExternalInput")
        return din[name]

    inp("xT0", (E, TL), f32)
    for l in (1, 2, 3):
        inp(f"wqkvT{l}", (P, 48 * E), bf16)        # packed [p][m][e][n]
        inp(f"wswiU{l}", (P, NC * E), bf16)        # [p][c][m0..7][n] packed
        inp(f"wswiG{l}", (P, NC * E), bf16)        # [p][c][m8..15][n] packed
        inp(f"woutT{l}", (P, NE * E), bf16)        # packed [p][m][e][n]
    inp("wqk4c", (P, NC * E), bf16)                # packed [q_h0,k_h0,q_h1,..]
    inp("wv4c", (2 * P, NE * TL), bf16)            # [vb, p, e, n]
    inp("wswiT4c", (P, NC * E), bf16)              # packed in chunk order
    inp("woutT4", (P, NE * NE * P), bf16)          # [p][c][mo][n]
    inp("cc", (P, T), bf16)
    inp("ss", (P, T), bf16)
    inp("cmask", (P, 4 * TL), bf16)
    inp("gate", (P, 1), f32)
    inp("ones_r", (P, P), f32r)
    inp("ones_b", (P, P), bf16)
    inp("sel4", (4, 4 * P), f32r)
    inp("klnb", (4, 1), f32)
    outT = nc.dram_tensor("outT", [E, TL], f32, kind="ExternalOutput")

    with tile.TileContext(nc) as tc:
        with (
            tc.tile_pool(name="const", bufs=1) as constp,
            tc.tile_pool(name="xres", bufs=10) as xp,
            tc.tile_pool(name="dram", bufs=1, space="DRAM") as dram,
        ):
            const = {}
            epsb = constp.tile([P, 1], f32, tag="epsb")
            nc.any.memset(epsb[:], EPS)
            const["epsb"] = epsb
            epsbdh = constp.tile([P, 1], f32, tag="epsbdh")
            nc.any.memset(epsbdh[:], DH * EPS)
            const["epsbdh"] = epsbdh
            oneb = constp.tile([P, 1], f32, tag="oneb")
            nc.any.memset(oneb[:], 1.0)
            const["oneb"] = oneb
            for nm, dt in (("cc", bf16), ("ss", bf16), ("cmask", bf16),
                           ("gate", f32), ("ones_r", f32r),
                           ("ones_b", bf16), ("sel4", f32r), ("klnb", f32)):
                t = constp.tile(list(din[nm].shape), dt, tag=nm)
                nc.sync.dma_start(t[:], din[nm].ap())
                const[nm] = t

            # load residual stream into SBUF once
            x_tiles = []
            for e in range(NE):
                xt = xp.tile([P, TL], f32, tag="x", bufs=10)
                nc.sync.dma_start(xt[:], din["xT0"].ap()[e * P:(e + 1) * P, :])
                x_tiles.append(xt)

            agx_in = dram.tile([E, TL], bf16, tag="agx", name="agx")
            if True:
                for l in (1, 2, 3):
                    ag_ins = [dram.tile([1, 2048], f32, tag=f"agi{l}_{g}",
                                        name=f"agi{l}_{g}") for g in range(2)]
                    ag_outs = [dram.tile([2, 2048], f32, tag=f"ago{l}_{g}",
                                         name=f"ago{l}_{g}") for g in range(2)]
                    x_tiles = _build_aft_layer(
                        tc, const, x_tiles, xp,
                        din[f"wqkvT{l}"], din[f"wswiU{l}"], din[f"wswiG{l}"],
                        din[f"woutT{l}"],
                        ag_ins, ag_outs,
                        x3_bf=(agx_in.opt() if l == 3 else None))

            agx_out_h = [dram.tile([E, TL], bf16, tag=f"agxo{h}",
                                   name=f"agxo{h}") for h in range(2)]
            for half in range(2):
                nc.gpsimd.collective_compute(
                    "AllGather", ALU.bypass, replica_groups=PAIRS,
                    ins=[agx_in.opt()[half * (E // 2):(half + 1) * (E // 2), :]],
                    outs=[agx_out_h[half].opt()])
            rs_in_q = [dram.tile([NE * P, TL], bf16, tag=f"rsi{j}",
                                 name=f"rsi{j}") for j in range(4)]
            rs_out_q = [dram.tile([4 * P, TL], bf16, tag=f"rso{j}",
                                  name=f"rso{j}") for j in range(4)]
            _build_tea(tc, const, x_tiles, din["wqk4c"], din["wv4c"],
                       din["wswiT4c"], din["woutT4"], agx_out_h,
                       rs_in_q, rs_out_q, outT)

    nc.compile()
    return nc


# --------------------------------------------------------------------------
# host-side sharding
# --------------------------------------------------------------------------

def _host_inputs(inputs):
    f = np.float32
    cos = np.ascontiguousarray(np.asarray(inputs['cos'], f)[:, 0, :].T)
    sin = np.ascontiguousarray(np.asarray(inputs['sin'], f)[:, 0, :].T)
    cc = np.concatenate([cos, cos], 0)
    ss = np.concatenate([sin, -sin], 0)
    cm = np.zeros((4, P, TL), f)
    kk = np.arange(P)[:, None]
    qq = np.arange(TL)[None, :]
    for j in range(4):
        cm[j] = np.where(P * j + kk <= qq, 0.0, -1e30)
    cmask = np.ascontiguousarray(cm.transpose(1, 0, 2).reshape(P, 4 * TL))
    ones_r = np.ones((P, P), f)
    ones_b = np.ones((P, P), BF)

    def tl(wT):
        # (K, M) -> tile layout (M, K): row-block m = [p, e, n] contiguous
        K, M = wT.shape
        return np.ascontiguousarray(
            wT.reshape(K // P, P, M // P, P).transpose(2, 1, 0, 3)
            .reshape(M, K))

    def pk(wT, perm=None):
        # (K, M) -> (P, (M/P)*K) packed: row p holds [m][e][n] contiguous,
        # so a G-tile DMA is one G*K-elem chunk per partition.
        K, M = wT.shape
        t = tl(wT).reshape(M // P, P, K).transpose(1, 0, 2)  # (P, m, K)
        if perm is not None:
            t = t[:, perm, :]
        return np.ascontiguousarray(t.reshape(P, (M // P) * K))

    sel4 = np.zeros((4, 4 * P), f)
    for i in range(4):
        sel4[i, i * P:(i + 1) * P] = 1.0
    klnb = np.array([[0.0], [0.0], [0.5 * np.log(DH)], [0.5 * np.log(DH)]], f)
    shared = {'cc': cc.astype(BF), 'ss': ss.astype(BF),
              'cmask': cmask.astype(BF), 'ones_r': ones_r,
              'ones_b': ones_b, 'sel4': sel4, 'klnb': klnb}
    for l in (1, 2, 3):
        shared[f'wqkvT{l}'] = pk(np.asarray(inputs[f'w_qkv{l}'], f).T).astype(BF)
        wswiT = np.asarray(inputs[f'w_swiglu{l}'], f).T   # (QKV, 2E)
        # [p][c][m][n] packing of each half: swiglu pass-1/2 c-tile loads
        for nm, half in (('wswiU', wswiT[:, :E]), ('wswiG', wswiT[:, E:])):
            shared[f'{nm}{l}'] = np.ascontiguousarray(
                half.reshape(NC, P, NE, P).transpose(1, 0, 2, 3)
                .reshape(P, NC * E)).astype(BF)
        shared[f'woutT{l}'] = pk(np.asarray(inputs[f'w_out{l}'], f).T).astype(BF)
    wout4T = np.asarray(inputs['w_out4'], f).T             # (E, E)
    # [p][c][mo][n] c-major packing for the c-outer TEA out-projection
    shared['woutT4'] = np.ascontiguousarray(
        wout4T.reshape(NE, P, NE, P).transpose(1, 0, 2, 3)
        .reshape(P, NE * E)).astype(BF)

    wq4 = np.asarray(inputs['w_qkv4'], f).T       # (E, 6144): per-head blocks
    wswi4 = np.asarray(inputs['w_swiglu4'], f).T  # (QKV, 2E)
    by_par = {}
    for par in range(2):
        hs = par * 8
        qk_cols = []
        for h in range(hs, hs + 8):     # interleaved [q_h, k_h] pairs
            for part in range(2):
                qk_cols.append(wq4[:, h * 3 * DH + part * DH:
                                   h * 3 * DH + (part + 1) * DH])
        v_cols = [wq4[:, h * 3 * DH + 2 * DH: h * 3 * DH + 3 * DH]
                  for h in range(hs, hs + 8)]
        kv = np.concatenate(v_cols, 1)             # (E, 1024)
        # wv4c layout [vb, p, e, n]: element = kv[128e + p, vb*512 + n]
        wv4c = np.ascontiguousarray(
            kv.reshape(NE, P, 2, TL).transpose(2, 1, 0, 3)
            .reshape(2 * P, NE * TL))
        # chunk-order permutation for the 4-way ReduceScatter pipeline
        swi_perm = [m for j in range(4)
                    for m in (2 * j, 2 * j + 1, 8 + 2 * j, 9 + 2 * j)]
        by_par[par] = {
            'wqk4c': pk(np.concatenate(qk_cols, 1)).astype(BF),
            'wv4c': wv4c.astype(BF),
            'wswiT4c': pk(np.ascontiguousarray(
                wswi4[hs * DH:(hs + 8) * DH, :]), perm=swi_perm).astype(BF),
            'gate': np.full((P, 1), float(par), f),
        }

    x = np.asarray(inputs['x'], f)
    in_maps = []
    for c in range(NCORES):
        b, par = c // 2, c % 2
        m = dict(shared)
        m.update(by_par[par])
        m['xT0'] = np.ascontiguousarray(x[b, par * TL:(par + 1) * TL, :].T)
        in_maps.append(m)
    return in_maps


_cached = {}


def kernel(**inputs):
    if 'nc' not in _cached:
        _cached['nc'] = build_program()
    nc = _cached['nc']
    in_maps = _host_inputs(inputs)
    trace = bool(int(os.environ.get('BASS_KERNEL_TRACE', '0')))
    res = run_bass_kernel_spmd(nc, in_maps, core_ids=list(range(NCORES)),
                               trace=trace)
    _cached['last_results'] = res
    out = np.zeros((4, T, E), np.float32)
    for c in range(NCORES):
        b, par = c // 2, c % 2
        out[b, par * TL:(par + 1) * TL, :] = res.results[c]['outT'].T
    return out


# revision 47
# speedup vs baseline: 1.1566x; 1.1566x over previous
"""TRN2 Bass kernel for nn_Block_82325933129820.

3x AFT blocks + 1 transformer (TEA) block, B=4 T=1024 E=1024 QKV=2048 H=16.

Sharding: 8 cores = 4 batch-pairs. Within a pair (even core, odd core):
  - AFT layers: token-split (even: tokens 0-511, odd: 512-1023), feature-major
    activations (channels on partitions, tokens on free dim). The cumsum runs
    as per-chunk tensor_tensor_scan along the free dim; cross-core carries
    travel via pair AllGathers and enter as the scan's `initial` value, gated
    to zero on even cores (with the denominator's +1e-6 folded in).
  - TEA: head-split (even: heads 0-7, odd: 8-15) over the full 1024 tokens.
    x3 is pair-AllGathered in bf16; attention is computed in S^T layout; the
    swiglu partial contraction is pair-ReduceScattered in bf16.

Precision/perf strategy: all GEMM weights are bf16 (half the HBM traffic);
PSUM accumulation is fp32. AFT intermediate activations (q/k/w/wv/yf) are
bf16 which enables DVE 2x modes; the cumsum scan recurrence is fp32
internally regardless. TEA attention internals stay fp32 (f32r matmuls at
full PE rate for N>=512). All reciprocals run on the scalar engine as
Exp(-Ln(x)); rsqrt(x) = Exp(-0.5*Ln(x)); sigmoid/silu via Exp with the
reciprocal folded into existing products; "+1"/"+eps" constants folded into
activation bias / scan initials. The AFT swiglu's first 8 output tiles
accumulate c-interleaved with the cumsum pipeline so the tensor engine
stays busy through the vector-heavy phase.
"""
import os
import sys
import numpy as np
import ml_dtypes

for _p in ('/opt/trn_rl_repo',):
    if _p not in sys.path:
        sys.path.insert(0, _p)

import concourse.bass as bass
import concourse.mybir as mybir
import concourse.tile as tile
from concourse import bacc
from concourse.bass_utils import run_bass_kernel_spmd

P = 128
TL = 512          # AFT tokens per core
E = 1024
QKV = 2048
T = 1024
DH = 128
NCORES = 8
NE = E // P       # 8
NC = QKV // P     # 16
EPS = float(np.finfo(np.float32).eps)
f32 = mybir.dt.float32
f32r = mybir.dt.float32r
bf16 = mybir.dt.bfloat16
AF = mybir.ActivationFunctionType
ALU = mybir.AluOpType
PAIRS = [[0, 1], [2, 3], [4, 5], [6, 7]]
BF = ml_dtypes.bfloat16


def _rsqrt(nc, pool, src_ps, scale, bias_ap, tag, ln_bufs=None):
    """rsqrt(src*scale + bias) = Exp(-0.5*Ln(.)). src_ps is PSUM (P, n)."""
    n = src_ps.shape[-1]
    tmp = pool.tile([P, n], f32, tag="lntmp", bufs=ln_bufs)
    nc.scalar.activation(tmp[:], src_ps[:], AF.Ln, scale=scale, bias=bias_ap)
    out = pool.tile([P, n], bf16, tag=tag)
    nc.scalar.activation(out[:], tmp[:], AF.Exp, scale=-0.5)
    return out


def _wgroup(nc, pool, wdram, m0, G, K=E, tag="wkg", bufs=None, name="wt",
            splits=2):
    """(P, G, K//P, P) bf16 weight group from host-packed (P, Mtiles*K) DRAM.

    Row p of the DRAM tensor holds tile-m-major data, so a G-tile load is
    one G*K*2-byte contiguous chunk per partition (fat DMA descriptors).
    The load is issued as `splits` partition-sliced dma_starts so several
    hardware queues stream concurrently (per-stream DMA bw is the limiter).
    """
    wt = pool.tile([P, G, K // P, P], bf16, tag=tag, bufs=bufs, name=name)
    step = P // splits
    for s in range(splits):
        lo, hi = s * step, (s + 1) * step
        nc.sync.dma_start(wt[lo:hi], wdram.ap()[lo:hi, m0 * K:(m0 + G) * K]
                          .rearrange("p (b a n) -> p b a n", b=G, n=P))
    return wt


def _build_aft_layer(tc, const, x_tiles, xp, wqkvT, wswiU, wswiG,
                     woutT, ag_ins, ag_outs, x3_bf=None):
    """One AFT layer, fully SBUF-resident activations.

    x_tiles: list of 8 (P, TL) f32 SBUF tiles (residual stream).
    Returns the new list of 8 x tiles (allocated from xp).
    If x3_bf is given (layer 3), also writes the bf16 output to that DRAM AP.
    """
    nc = tc.nc
    ones_b = const["ones_b"]
    gate_col = const["gate"]

    with (
        tc.tile_pool(name="a_sc", bufs=2) as scp,
        tc.tile_pool(name="a_k", bufs=NC) as kp,
        tc.tile_pool(name="a_q", bufs=NC) as qp,
        tc.tile_pool(name="a_ww", bufs=NC) as wwp,
        tc.tile_pool(name="a_yf", bufs=NC) as yfp,
        tc.tile_pool(name="a_cc", bufs=8) as ccp,
        tc.tile_pool(name="a_xn", bufs=NE) as xnp,
    ):
        yf_t = [None] * NC
        w_t = [None] * NC
        wv_t = [None] * NC
        with (
            tc.tile_pool(name="a_w8", bufs=4) as wp,
            tc.tile_pool(name="a_ld", bufs=4) as sbp,
            tc.tile_pool(name="a_ps", bufs=4, space="PSUM") as ps,
            tc.tile_pool(name="a_ps2", bufs=1, space="PSUM") as ps2,
        ):
            # ---- rms(x) ----
            xsq = []
            for e in range(NE):
                t = sbp.tile([P, TL], bf16, tag="sq", bufs=NE)
                nc.gpsimd.tensor_tensor(t[:], x_tiles[e][:], x_tiles[e][:],
                                        ALU.mult)
                xsq.append(t)
            sumsq = ps2.tile([P, TL], f32, tag="xsumsq")
            for e in range(NE):
                nc.tensor.matmul(sumsq[:], ones_b[:], xsq[e][:],
                                 start=(e == 0), stop=(e == NE - 1))
            xscale = _rsqrt(nc, scp, sumsq, 1.0 / E, const["epsb"][:],
                            "scale")
            xn = []
            for e in range(NE):
                t = xnp.tile([P, TL], bf16, tag="xn")
                nc.gpsimd.tensor_tensor(t[:], x_tiles[e][:], xscale[:],
                                        ALU.mult)
                xn.append(t)

            def qkv_group(mt0, gblk):
                """Load G=4 qkv weight tiles, return the group tile."""
                return _wgroup(nc, wp, wqkvT, mt0 + 4 * gblk, 4, tag="wk4",
                               splits=1)

            def qkv_acc(wt, b):
                acc = ps.tile([P, TL], f32, tag="mm", name="acc")
                for e in range(NE):
                    nc.tensor.matmul(acc[:], wt[:, b, e, :], xn[e][:],
                                     start=(e == 0), stop=(e == NE - 1))
                return acc

            # ---- k tiles (SBUF-resident bf16); k weight tiles are m 16..31
            k_sb = [None] * NC
            ksq = [None] * NC
            for gblk in range(4):
                wt = qkv_group(16, gblk)
                for b in range(4):
                    c = 4 * gblk + b
                    acc = qkv_acc(wt, b)
                    kt = kp.tile([P, TL], bf16, tag="k")
                    nc.scalar.copy(kt[:], acc[:])
                    k_sb[c] = kt
                    sq = sbp.tile([P, TL], bf16, tag="sq", bufs=NE)
                    nc.gpsimd.tensor_tensor(sq[:], kt[:], kt[:], ALU.mult)
                    ksq[c] = sq
            ksumsq = ps2.tile([P, TL], f32, tag="ksumsq")
            for c in range(NC):
                nc.tensor.matmul(ksumsq[:], ones_b[:], ksq[c][:],
                                 start=(c == 0), stop=(c == NC - 1))
            kscale = _rsqrt(nc, scp, ksumsq, 1.0 / QKV, const["epsb"][:],
                            "scale")

            # ---- v matmuls + w/wv + carries (2 groups of 8); v is m 32..47
            for g in range(2):
                for gblk in range(2 * g, 2 * g + 2):
                    wt = qkv_group(32, gblk)
                    for b in range(4):
                        c = 4 * gblk + b
                        kn = sbp.tile([P, TL], bf16, tag="kn", bufs=3)
                        nc.gpsimd.tensor_tensor(kn[:], k_sb[c][:],
                                                kscale[:], ALU.mult)
                        w = wwp.tile([P, TL], bf16, tag="w")
                        cw_col = ccp.tile([P, 1], f32, tag="cwc")
                        nc.scalar.activation(w[:], kn[:], AF.Exp,
                                             accum_out=cw_col[:])
                        acc = qkv_acc(wt, b)
                        wv = wwp.tile([P, TL], bf16, tag="wv")
                        cwv_col = ccp.tile([P, 1], f32, tag="cwvc")
                        nc.vector.scalar_tensor_tensor(
                            wv[:], acc[:], 0.0, w[:], ALU.bypass, ALU.mult,
                            accum_out=cwv_col[:])
                        j = c - 8 * g
                        nc.sync.dma_start(
                            ag_ins[g].opt()[:, j * P:(j + 1) * P]
                            .rearrange("o (p q) -> p (o q)", p=P),
                            cwv_col[:])
                        nc.sync.dma_start(
                            ag_ins[g].opt()[:, 1024 + j * P:1024 + (j + 1) * P]
                            .rearrange("o (p q) -> p (o q)", p=P),
                            cw_col[:])
                        w_t[c] = w
                        wv_t[c] = wv
                nc.gpsimd.collective_compute(
                    "AllGather", ALU.bypass, replica_groups=PAIRS,
                    ins=[ag_ins[g].opt()], outs=[ag_outs[g].opt()])

            # ---- q tiles (SBUF-resident bf16); q is m 0..15 ----
            q_sb = [None] * NC
            qsq = [None] * NC
            for gblk in range(4):
                wt = qkv_group(0, gblk)
                for b in range(4):
                    c = 4 * gblk + b
                    acc = qkv_acc(wt, b)
                    qt = qp.tile([P, TL], bf16, tag="q")
                    nc.scalar.copy(qt[:], acc[:])
                    q_sb[c] = qt
                    sq = sbp.tile([P, TL], bf16, tag="sq", bufs=NE)
                    nc.gpsimd.tensor_tensor(sq[:], qt[:], qt[:], ALU.mult)
                    qsq[c] = sq
            qsumsq = ps2.tile([P, TL], f32, tag="qsumsq")
            for c in range(NC):
                nc.tensor.matmul(qsumsq[:], ones_b[:], qsq[c][:],
                                 start=(c == 0), stop=(c == NC - 1))
            qscale = _rsqrt(nc, scp, qsumsq, 1.0 / QKV, const["epsb"][:],
                            "scale")

        # ---- phase B (scans etc.) interleaved with swiglu pass 1 ----
        with (
            tc.tile_pool(name="a_sw", bufs=2) as swp,
            tc.tile_pool(name="a_pb", bufs=2) as pbp,
            tc.tile_pool(name="a_u", bufs=NE) as up,
            tc.tile_pool(name="a_mt", bufs=NE) as mtp,
            tc.tile_pool(name="a_pss", bufs=8, space="PSUM") as pss,
        ):
            sacc = [None] * NE
            for g in range(2):
                cwv_raw = ccp.tile([P, 8], f32, tag="cwvr")
                nc.sync.dma_start(
                    cwv_raw[:], ag_outs[g].opt()[0:1, 0:1024]
                    .rearrange("o (c p) -> p (o c)", p=P))
                cw_raw = ccp.tile([P, 8], f32, tag="cwr")
                nc.sync.dma_start(
                    cw_raw[:], ag_outs[g].opt()[0:1, 1024:2048]
                    .rearrange("o (c p) -> p (o c)", p=P))
                cwv_g = ccp.tile([P, 8], f32, tag="cwvg")
                nc.vector.tensor_scalar(cwv_g[:], cwv_raw[:],
                                        gate_col[:], None, ALU.mult)
                # denominator carry gets the +1e-6 folded in
                cw_g = ccp.tile([P, 8], f32, tag="cwg")
                nc.vector.tensor_scalar(cw_g[:], cw_raw[:],
                                        gate_col[:], 1e-6,
                                        ALU.mult, ALU.add)
                for c in range(8 * g, 8 * g + 8):
                    j = c - 8 * g
                    sw = pbp.tile([P, TL], bf16, tag="sw")
                    nc.vector.tensor_tensor_scan(
                        sw[:], wv_t[c][:], wv_t[c][:], cwv_g[:, j:j + 1],
                        ALU.add, ALU.bypass)
                    sw2 = pbp.tile([P, TL], bf16, tag="sw2")
                    nc.vector.tensor_tensor_scan(
                        sw2[:], w_t[c][:], w_t[c][:], cw_g[:, j:j + 1],
                        ALU.add, ALU.bypass)
                    qn = pbp.tile([P, TL], bf16, tag="qn")
                    nc.gpsimd.tensor_tensor(qn[:], q_sb[c][:], qscale[:],
                                            ALU.mult)
                    et = pbp.tile([P, TL], bf16, tag="et")
                    nc.scalar.activation(et[:], qn[:], AF.Exp, scale=-1.0)
                    # dd = (et + 1) * sw2   (sw2 already carries the +1e-6)
                    dd = pbp.tile([P, TL], bf16, tag="dd")
                    nc.vector.scalar_tensor_tensor(
                        dd[:], et[:], 1.0, sw2[:], ALU.add, ALU.mult)
                    lnd = pbp.tile([P, TL], bf16, tag="lnd")
                    nc.scalar.activation(lnd[:], dd[:], AF.Ln)
                    rr = pbp.tile([P, TL], bf16, tag="rr")
                    nc.scalar.activation(rr[:], lnd[:], AF.Exp, scale=-1.0)
                    yf = yfp.tile([P, TL], bf16, tag="yf")
                    nc.vector.tensor_tensor(yf[:], sw[:], rr[:], ALU.mult)
                    yf_t[c] = yf
                    # swiglu pass 1 (u half, m=0..7), c-interleaved
                    if c % 2 == 0:
                        w1 = _wgroup(nc, swp, wswiU, c, 2, tag="w1",
                                     name="w1")
                    for m in range(NE):
                        if c == 0:
                            sacc[m] = pss.tile([P, TL], f32, tag="sacc", name="sacc")
                        nc.tensor.matmul(sacc[m][:], w1[:, c % 2, m, :],
                                         yf[:],
                                         start=(c == 0), stop=(c == NC - 1))

            # drain u, then swiglu pass 2 (g half, m=8..15), c-outer
            u_sb = [None] * NE
            for m in range(NE):
                ut = up.tile([P, TL], bf16, tag="u")
                nc.scalar.copy(ut[:], sacc[m][:])
                u_sb[m] = ut
            sacc2 = [None] * NE
            for c in range(NC):
                if c % 2 == 0:
                    w2 = _wgroup(nc, swp, wswiG, c, 2, tag="w2", name="w2")
                for m in range(NE):
                    if c == 0:
                        sacc2[m] = pss.tile([P, TL], f32, tag="sacc", name="sacc2")
                    nc.tensor.matmul(sacc2[m][:], w2[:, c % 2, m, :],
                                     yf_t[c][:],
                                     start=(c == 0), stop=(c == NC - 1))
            # silu: m = u * g / (1 + exp(-g))
            m_t = [None] * NE
            for m in range(NE):
                eg = pbp.tile([P, TL], bf16, tag="eg")
                nc.scalar.activation(eg[:], sacc2[m][:], AF.Exp, scale=-1.0)
                lnd = pbp.tile([P, TL], bf16, tag="lnd")
                nc.scalar.activation(lnd[:], eg[:], AF.Ln,
                                     bias=const["oneb"][:])
                rr = pbp.tile([P, TL], bf16, tag="rr")
                nc.scalar.activation(rr[:], lnd[:], AF.Exp, scale=-1.0)
                pug = pbp.tile([P, TL], bf16, tag="pug")
                nc.vector.tensor_tensor(pug[:], u_sb[m][:], sacc2[m][:],
                                        ALU.mult)
                mt = mtp.tile([P, TL], bf16, tag="mt")
                nc.gpsimd.tensor_tensor(mt[:], pug[:], rr[:], ALU.mult)
                m_t[m] = mt

            # ---- out-proj + residual (SBUF resident) ----
            new_x = []
            with tc.tile_pool(name="a_w8b", bufs=2) as wpb:
                for mo in range(NE):
                    if mo % 4 == 0:
                        wo = _wgroup(nc, wpb, woutT, mo, 4, tag="wo",
                                     name="wo")
                    acc = pss.tile([P, TL], f32, tag="sacc", name="oacc")
                    for c in range(NE):
                        nc.tensor.matmul(acc[:], wo[:, mo % 4, c, :],
                                         m_t[c][:],
                                         start=(c == 0), stop=(c == NE - 1))
                    xo = xp.tile([P, TL], f32, tag="x", bufs=10)
                    nc.vector.tensor_tensor(xo[:], acc[:], x_tiles[mo][:],
                                            ALU.add)
                    new_x.append(xo)
                    if x3_bf is not None:
                        xob = pbp.tile([P, TL], bf16, tag="xob")
                        nc.scalar.copy(xob[:], xo[:])
                        nc.sync.dma_start(
                            x3_bf[mo * P:(mo + 1) * P, :], xob[:])
    return new_x


def _build_tea(tc, const, x_tiles, wqk4c, wv4c, wswiT4c, woutT4,
               agx_out_h, rs_in_q, rs_out_q, outT):
    nc = tc.nc
    ones_r = const["ones_r"]
    cc_t, ss_t, cm_t = const["cc"], const["ss"], const["cmask"]
    HL = 8

    with (
        tc.tile_pool(name="t_yt", bufs=2 * HL) as ytp,
        tc.tile_pool(name="t_sc", bufs=2) as scp,
        tc.tile_pool(name="t_ps", bufs=2, space="PSUM") as ps,
        tc.tile_pool(name="t_ps2", bufs=2, space="PSUM") as ps2,
        tc.tile_pool(name="t_xn", bufs=2 * NE) as xnp,
        tc.tile_pool(name="t_v", bufs=16) as vp,
    ):
        with tc.tile_pool(name="t_t", bufs=3) as sbp:
            # ---- rms(x3) (x3 arrives bf16 via the pair AllGather) ----
            xn = [[None] * NE for _ in range(2)]
            for tch in range(2):
                def _x3_ap(tch, e):
                    half, er = e // 4, e % 4
                    return agx_out_h[half].opt()[
                        tch * (E // 2) + er * P:tch * (E // 2) + (er + 1) * P, :]

                xt3s = []
                for e in range(NE):
                    xt3 = sbp.tile([P, TL], bf16, tag="xt3", bufs=NE)
                    nc.sync.dma_start(xt3[:], _x3_ap(tch, e))
                    xt3s.append(xt3)
                sumsq = ps2.tile([P, TL], f32, tag="sumsq")
                for e in range(NE):
                    xsq = sbp.tile([P, TL], bf16, tag="sq")
                    nc.gpsimd.tensor_tensor(xsq[:], xt3s[e][:], xt3s[e][:],
                                            ALU.mult)
                    nc.tensor.matmul(sumsq[:], const["ones_b"][:], xsq[:],
                                     start=(e == 0), stop=(e == NE - 1))
                xscale = _rsqrt(nc, scp, sumsq, 1.0 / E, const["epsb"][:],
                                "xscale", ln_bufs=2)
                for e in range(NE):
                    t = xnp.tile([P, TL], bf16, tag="xn")
                    nc.vector.tensor_tensor(t[:], xt3s[e][:], xscale[:],
                                            ALU.mult)
                    xn[tch][e] = t

            # ---- V (token-major) ----
            V = [[None] * 2 for _ in range(8)]
            with tc.tile_pool(name="t_vw", bufs=2) as vwp:
                for vb in range(2):
                    vw = vwp.tile([P, NE, TL], bf16, tag="vw")
                    nc.sync.dma_start(
                        vw[:],
                        wv4c.ap()[vb * P:(vb + 1) * P, :]
                        .rearrange("p (a n) -> p a n", n=TL))
                    for ttile in range(8):
                        tch, toff = ttile // 4, (ttile % 4) * P
                        acc = ps.tile([P, TL], f32, tag="mm")
                        for e in range(NE):
                            nc.tensor.matmul(
                                acc[:], xn[tch][e][:, toff:toff + P],
                                vw[:, e, :],
                                start=(e == 0), stop=(e == NE - 1))
                        vt = vp.tile([P, TL], bf16, tag="V")
                        nc.scalar.copy(vt[:], acc[:])
                        V[ttile][vb] = vt

        # ---- per-head rope/rms + attention ----
        yT = [[None] * 2 for _ in range(HL)]
        with (
            tc.tile_pool(name="t_qk", bufs=8) as qkp,
            tc.tile_pool(name="t_es", bufs=8) as esp,
            tc.tile_pool(name="t_w8", bufs=3) as wp,
            tc.tile_pool(name="t_at", bufs=2) as sba,
            tc.tile_pool(name="t_psa", bufs=2, space="PSUM") as psa,
            tc.tile_pool(name="t_psd", bufs=1, space="PSUM") as psd,
        ):
            sel4 = const["sel4"]

            def qk_phase(h):
                """QK matmuls + rope + rms-scale for head h; returns
                (qn_h, kn_h) f32r SBUF tiles."""
                qn_h = [None] * 2
                kn_h = [None] * 2
                sites = []
                coll = scp.tile([4, TL], f32, tag="coll", bufs=2,
                                name="coll")
                wqk = _wgroup(nc, wp, wqk4c, 2 * h, 2, tag="wqk", name="wqk")
                for wi, out_list in enumerate((qn_h, kn_h)):
                    for tch in range(2):
                        acc = ps.tile([P, TL], f32, tag="mm", name="acc")
                        for e in range(NE):
                            nc.tensor.matmul(acc[:], wqk[:, wi, e, :],
                                             xn[tch][e][:],
                                             start=(e == 0),
                                             stop=(e == NE - 1))
                        zsq = sba.tile([P, TL], f32r, tag="sq", name="zsq")
                        nc.scalar.activation(zsq[:], acc[:], AF.Square)
                        sq_ps = ps2.tile([1, TL], f32, tag="sumsq",
                                         name="sq_ps")
                        nc.tensor.matmul(sq_ps[:], ones_r[:, 0:1], zsq[:],
                                         start=True, stop=True)
                        r = 2 * wi + tch
                        srow = scp.tile([1, TL], f32, tag="srow", bufs=3,
                                        name="srow")
                        nc.scalar.copy(srow[:], sq_ps[:])
                        nc.sync.dma_start(coll[r:r + 1, :], srow[:])
                        tsl = slice(tch * TL, (tch + 1) * TL)
                        tmp1 = sba.tile([P, TL], f32, tag="tmp1",
                                        name="tmp1")
                        nc.vector.tensor_tensor(tmp1[:], acc[:],
                                                cc_t[:, tsl], ALU.mult)
                        cross = sba.tile([P, TL], f32, tag="cross",
                                         name="cross")
                        nc.vector.tensor_tensor(cross[:64, :], acc[64:, :],
                                                ss_t[:64, tsl], ALU.mult)
                        nc.vector.tensor_tensor(cross[64:, :], acc[:64, :],
                                                ss_t[64:, tsl], ALU.mult)
                        zrope = sba.tile([P, TL], f32, tag="zrope",
                                         bufs=6, name="zrope")
                        nc.gpsimd.tensor_tensor(zrope[:], tmp1[:], cross[:],
                                                ALU.add)
                        sites.append((r, zrope, out_list, tch))
                # one Ln + one Exp for all 4 sites of this head.
                lnc = scp.tile([4, TL], f32, tag="lnc", bufs=2, name="lnc")
                nc.scalar.activation(lnc[:], coll[:], AF.Ln,
                                     bias=const["epsbdh"][0:4, :])
                esc = scp.tile([4, TL], f32r, tag="esc", bufs=2, name="esc")
                nc.scalar.activation(esc[:], lnc[:], AF.Exp, scale=-0.5,
                                     bias=const["klnb"][:])
                for r, zrope, out_list, tch in sites:
                    sc_ps = ps.tile([P, TL], f32, tag="mm", name="sc_ps")
                    nc.tensor.matmul(sc_ps[:], sel4[:, r * P:(r + 1) * P],
                                     esc[:], start=True, stop=True)
                    zn = qkp.tile([P, TL], f32r, tag="zn", name="zn")
                    nc.vector.tensor_tensor(zn[:], zrope[:], sc_ps[:],
                                            ALU.mult)
                    out_list[tch] = zn
                return qn_h, kn_h

            def attn_phase(h, qn_h, kn_h):
                for qc in range(2):
                    denom = psd.tile([P, TL], f32, tag="denom")
                    ytil = psd.tile([P, TL], f32, tag="ytil")
                    nkt = 4 * (qc + 1)
                    for kt in range(nkt):
                        tch_k, koff = kt // 4, (kt % 4) * P
                        sT = psa.tile([P, TL], f32, tag="sT")
                        nc.tensor.matmul(sT[:],
                                         kn_h[tch_k][:, koff:koff + P],
                                         qn_h[qc][:], start=True, stop=True)
                        es = esp.tile([P, TL], bf16, tag="es")
                        j = kt - 4 * qc
                        if j >= 0:
                            sm = sba.tile([P, TL], f32, tag="sm")
                            nc.vector.tensor_tensor(
                                sm[:], sT[:], cm_t[:, j * TL:(j + 1) * TL],
                                ALU.add)
                            nc.scalar.activation(es[:], sm[:], AF.Exp)
                        else:
                            nc.scalar.activation(es[:], sT[:], AF.Exp)
                        nc.tensor.matmul(denom[:], const["ones_b"][:], es[:],
                                         start=(kt == 0),
                                         stop=(kt == nkt - 1))
                        nc.tensor.matmul(
                            ytil[:],
                            V[kt][h // 4][:, (h % 4) * P:(h % 4 + 1) * P],
                            es[:], start=(kt == 0), stop=(kt == nkt - 1))
                    lnr = sba.tile([P, TL], f32, tag="lnr")
                    nc.scalar.activation(lnr[:], denom[:], AF.Ln)
                    rr = sba.tile([P, TL], f32, tag="arr")
                    nc.scalar.activation(rr[:], lnr[:], AF.Exp, scale=-1.0)
                    yt = ytp.tile([P, TL], bf16, tag="yT")
                    nc.vector.tensor_tensor(yt[:], ytil[:], rr[:], ALU.mult)
                    yT[h][qc] = yt

            # software-pipeline: head h's norm-collection latency hides
            # under head h-1's attention matmuls
            pend = None
            for h in range(HL):
                qk = qk_phase(h)
                if pend is not None:
                    attn_phase(pend[0], *pend[1])
                pend = (h, qk)
            attn_phase(pend[0], *pend[1])

        # ---- partial swiglu, 4 ReduceScatter chunks pipelined ----
        # chunk j covers m-tiles {2j, 2j+1, 8+2j, 8+2j+1} (u-pair + g-pair);
        # wswiT4c is host-packed in exactly this consumption order.
        with (
            tc.tile_pool(name="t_w8s", bufs=2) as wps,
            tc.tile_pool(name="t_pug", bufs=4) as pugp,
        ):
            for j in range(4):
                wt = _wgroup(nc, wps, wswiT4c, 4 * j, 4, tag="ws",
                             name="ws")
                for s in range(4):
                    for tch in range(2):
                        acc = ps.tile([P, TL], f32, tag="mm")
                        for kk in range(HL):
                            nc.tensor.matmul(acc[:], wt[:, s, kk, :],
                                             yT[kk][tch][:],
                                             start=(kk == 0),
                                             stop=(kk == HL - 1))
                        pug = pugp.tile([P, TL], bf16, tag="pug")
                        nc.scalar.copy(pug[:], acc[:])
                        nc.sync.dma_start(
                            rs_in_q[j].opt()[tch * 4 * P + s * P:
                                             tch * 4 * P + (s + 1) * P, :],
                            pug[:])
                nc.gpsimd.collective_compute(
                    "ReduceScatter", ALU.add, replica_groups=PAIRS,
                    ins=[rs_in_q[j].opt()], outs=[rs_out_q[j].opt()])

        # ---- silu + out-proj (c-outer, overlaps RS chunks) + residual ----
        with (
            tc.tile_pool(name="t_mt", bufs=NE) as mtp,
            tc.tile_pool(name="t_w8o", bufs=1) as wpo,
            tc.tile_pool(name="t_t4", bufs=2) as sb4,
            tc.tile_pool(name="t_pso", bufs=4, space="PSUM") as pso,
        ):
            woc = wpo.tile([P, NE, NE, P], bf16, tag="woc", name="woc")
            nc.sync.dma_start(woc[:], woutT4.ap()
                              .rearrange("p (c b n) -> p c b n", c=NE, n=P))
            m_t = [None] * NE
            for j in range(4):
                for i in range(2):
                    c = 2 * j + i
                    ut = sb4.tile([P, TL], bf16, tag="u4")
                    nc.sync.dma_start(
                        ut[:], rs_out_q[j].opt()[i * P:(i + 1) * P, :])
                    gt = sb4.tile([P, TL], bf16, tag="g4")
                    nc.sync.dma_start(
                        gt[:],
                        rs_out_q[j].opt()[(2 + i) * P:(3 + i) * P, :])
                    eg = sb4.tile([P, TL], bf16, tag="eg4")
                    nc.scalar.activation(eg[:], gt[:], AF.Exp, scale=-1.0)
                    lnd = sb4.tile([P, TL], bf16, tag="lnd4")
                    nc.scalar.activation(lnd[:], eg[:], AF.Ln,
                                         bias=const["oneb"][:])
                    rr = sb4.tile([P, TL], bf16, tag="rr4")
                    nc.scalar.activation(rr[:], lnd[:], AF.Exp, scale=-1.0)
                    pug = sb4.tile([P, TL], bf16, tag="pug4")
                    nc.gpsimd.tensor_tensor(pug[:], ut[:], gt[:], ALU.mult)
                    mt = mtp.tile([P, TL], bf16, tag="mt4")
                    nc.vector.tensor_tensor(mt[:], pug[:], rr[:], ALU.mult)
                    m_t[c] = mt
            oacc = [None] * NE
            for half in range(2):
                for c in range(NE):
                    for mo in range(4 * half, 4 * half + 4):
                        if c == 0:
                            oacc[mo] = pso.tile([P, TL], f32, tag="oacc",
                                                name="oacc")
                        nc.tensor.matmul(oacc[mo][:], woc[:, c, mo, :],
                                         m_t[c][:],
                                         start=(c == 0), stop=(c == NE - 1))
                for mo in range(4 * half, 4 * half + 4):
                    xo = sb4.tile([P, TL], f32, tag="xo4")
                    nc.vector.tensor_tensor(xo[:], oacc[mo][:],
                                            x_tiles[mo][:], ALU.add)
                    nc.sync.dma_start(outT.ap()[mo * P:(mo + 1) * P, :],
                                      xo[:])


class _Bacc(bacc.Bacc):
    """Bacc with the combined ln+exp activation table given priority.

    The act-table insertion pass assigns each activation the first table
    in the list that contains its function; the default act_info order
    makes Exp resolve to `exp_and_others` and Ln to `natural_log`, so a
    kernel that alternates Exp/Ln (reciprocals, rsqrt) reloads the table
    on nearly every call (~1.3us each). Putting
    `natural_log_exp_and_others` first lets Exp/Ln/Square/Copy all share
    one resident table.
    """

    def insert_act_table_loads(self):
        import bass_rust as _bass_rust
        from concourse.hw_specs import get_activation_tables
        has_activation = any(
            isinstance(i, mybir.InstActivation)
            for b in self.main_func.blocks
            for i in b.instructions
        )
        if not has_activation:
            return
        steer = {AF.Exp, AF.Ln, AF.Square, AF.Copy}
        tables = [
            (nm, set(fns) if nm == 'natural_log_exp_and_others'
             else set(fns) - steer)
            for nm, fns in get_activation_tables(self.m.arch).items()
        ]
        _bass_rust.insert_act_table_loads(self, tables)


def build_program():
    nc = _Bacc("TRN2", target_bir_lowering=False, debug=False,
               num_devices=NCORES)

    din = {}

    def inp(name, shape, dt):
        din[name] = nc.dram_tensor(name, list(shape), dt,
                                   kind="ExternalInput")
        return din[name]

    inp("xT0", (E, TL), f32)
    for l in (1, 2, 3):
        inp(f"wqkvT{l}", (P, 48 * E), bf16)        # packed [p][m][e][n]
        inp(f"wswiU{l}", (P, NC * E), bf16)        # [p][c][m0..7][n] packed
        inp(f"wswiG{l}", (P, NC * E), bf16)        # [p][c][m8..15][n] packed
        inp(f"woutT{l}", (P, NE * E), bf16)        # packed [p][m][e][n]
    inp("wqk4c", (P, NC * E), bf16)                # packed [q_h0,k_h0,q_h1,..]
    inp("wv4c", (2 * P, NE * TL), bf16)            # [vb, p, e, n]
    inp("wswiT4c", (P, NC * E), bf16)              # packed in chunk order
    inp("woutT4", (P, NE * NE * P), bf16)          # [p][c][mo][n]
    inp("cc", (P, T), bf16)
    inp("ss", (P, T), bf16)
    inp("cmask", (P, 4 * TL), bf16)
    inp("gate", (P, 1), f32)
    inp("ones_r", (P, P), f32r)
    inp("ones_b", (P, P), bf16)
    inp("sel4", (4, 4 * P), f32r)
    inp("klnb", (4, 1), f32)
    outT = nc.dram_tensor("outT", [E, TL], f32, kind="ExternalOutput")

    with tile.TileContext(nc) as tc:
        with (
            tc.tile_pool(name="const", bufs=1) as constp,
            tc.tile_pool(name="xres", bufs=10) as xp,
            tc.tile_pool(name="dram", bufs=1, space="DRAM") as dram,
        ):
            const = {}
            epsb = constp.tile([P, 1], f32, tag="epsb")
            nc.any.memset(epsb[:], EPS)
            const["epsb"] = epsb
            epsbdh = constp.tile([P, 1], f32, tag="epsbdh")
            nc.any.memset(epsbdh[:], DH * EPS)
            const["epsbdh"] = epsbdh
            oneb = constp.tile([P, 1], f32, tag="oneb")
            nc.any.memset(oneb[:], 1.0)
            const["oneb"] = oneb
            for nm, dt in (("cc", bf16), ("ss", bf16), ("cmask", bf16),
                           ("gate", f32), ("ones_r", f32r),
                           ("ones_b", bf16), ("sel4", f32r), ("klnb", f32)):
                t = constp.tile(list(din[nm].shape), dt, tag=nm)
                nc.sync.dma_start(t[:], din[nm].ap())
                const[nm] = t

            # load residual stream into SBUF once
            x_tiles = []
            for e in range(NE):
                xt = xp.tile([P, TL], f32, tag="x", bufs=10)
                nc.sync.dma_start(xt[:], din["xT0"].ap()[e * P:(e + 1) * P, :])
                x_tiles.append(xt)

            agx_in = dram.tile([E, TL], bf16, tag="agx", name="agx")
            if True:
                for l in (1, 2, 3):
                    ag_ins = [dram.tile([1, 2048], f32, tag=f"agi{l}_{g}",
                                        name=f"agi{l}_{g}") for g in range(2)]
                    ag_outs = [dram.tile([2, 2048], f32, tag=f"ago{l}_{g}",
                                         name=f"ago{l}_{g}") for g in range(2)]
                    x_tiles = _build_aft_layer(
                        tc, const, x_tiles, xp,
                        din[f"wqkvT{l}"], din[f"wswiU{l}"], din[f"wswiG{l}"],
                        din[f"woutT{l}"],
                        ag_ins, ag_outs,
                        x3_bf=(agx_in.opt() if l == 3 else None))

            agx_out_h = [dram.tile([E, TL], bf16, tag=f"agxo{h}",
                                   name=f"agxo{h}") for h in range(2)]
            for half in range(2):
                nc.gpsimd.collective_compute(
                    "AllGather", ALU.bypass, replica_groups=PAIRS,
                    ins=[agx_in.opt()[half * (E // 2):(half + 1) * (E // 2), :]],
                    outs=[agx_out_h[half].opt()])
            rs_in_q = [dram.tile([NE * P, TL], bf16, tag=f"rsi{j}",
                                 name=f"rsi{j}") for j in range(4)]
            rs_out_q = [dram.tile([4 * P, TL], bf16, tag=f"rso{j}",
                                  name=f"rso{j}") for j in range(4)]
            _build_tea(tc, const, x_tiles, din["wqk4c"], din["wv4c"],
                       din["wswiT4c"], din["woutT4"], agx_out_h,
                       rs_in_q, rs_out_q, outT)

    nc.compile()
    return nc


# --------------------------------------------------------------------------
# host-side sharding
# --------------------------------------------------------------------------

def _host_inputs(inputs):
    f = np.float32
    cos = np.ascontiguousarray(np.asarray(inputs['cos'], f)[:, 0, :].T)
    sin = np.ascontiguousarray(np.asarray(inputs['sin'], f)[:, 0, :].T)
    cc = np.concatenate([cos, cos], 0)
    ss = np.concatenate([sin, -sin], 0)
    cm = np.zeros((4, P, TL), f)
    kk = np.arange(P)[:, None]
    qq = np.arange(TL)[None, :]
    for j in range(4):
        cm[j] = np.where(P * j + kk <= qq, 0.0, -1e30)
    cmask = np.ascontiguousarray(cm.transpose(1, 0, 2).reshape(P, 4 * TL))
    ones_r = np.ones((P, P), f)
    ones_b = np.ones((P, P), BF)

    def tl(wT):
        # (K, M) -> tile layout (M, K): row-block m = [p, e, n] contiguous
        K, M = wT.shape
        return np.ascontiguousarray(
            wT.reshape(K // P, P, M // P, P).transpose(2, 1, 0, 3)
            .reshape(M, K))

    def pk(wT, perm=None):
        # (K, M) -> (P, (M/P)*K) packed: row p holds [m][e][n] contiguous,
        # so a G-tile DMA is one G*K-elem chunk per partition.
        K, M = wT.shape
        t = tl(wT).reshape(M // P, P, K).transpose(1, 0, 2)  # (P, m, K)
        if perm is not None:
            t = t[:, perm, :]
        return np.ascontiguousarray(t.reshape(P, (M // P) * K))

    sel4 = np.zeros((4, 4 * P), f)
    for i in range(4):
        sel4[i, i * P:(i + 1) * P] = 1.0
    klnb = np.array([[0.0], [0.0], [0.5 * np.log(DH)], [0.5 * np.log(DH)]], f)
    shared = {'cc': cc.astype(BF), 'ss': ss.astype(BF),
              'cmask': cmask.astype(BF), 'ones_r': ones_r,
              'ones_b': ones_b, 'sel4': sel4, 'klnb': klnb}
    for l in (1, 2, 3):
        shared[f'wqkvT{l}'] = pk(np.asarray(inputs[f'w_qkv{l}'], f).T).astype(BF)
        wswiT = np.asarray(inputs[f'w_swiglu{l}'], f).T   # (QKV, 2E)
        # [p][c][m][n] packing of each half: swiglu pass-1/2 c-tile loads
        for nm, half in (('wswiU', wswiT[:, :E]), ('wswiG', wswiT[:, E:])):
            shared[f'{nm}{l}'] = np.ascontiguousarray(
                half.reshape(NC, P, NE, P).transpose(1, 0, 2, 3)
                .reshape(P, NC * E)).astype(BF)
        shared[f'woutT{l}'] = pk(np.asarray(inputs[f'w_out{l}'], f).T).astype(BF)
    wout4T = np.asarray(inputs['w_out4'], f).T             # (E, E)
    # [p][c][mo][n] c-major packing for the c-outer TEA out-projection
    shared['woutT4'] = np.ascontiguousarray(
        wout4T.reshape(NE, P, NE, P).transpose(1, 0, 2, 3)
        .reshape(P, NE * E)).astype(BF)

    wq4 = np.asarray(inputs['w_qkv4'], f).T       # (E, 6144): per-head blocks
    wswi4 = np.asarray(inputs['w_swiglu4'], f).T  # (QKV, 2E)
    by_par = {}
    for par in range(2):
        hs = par * 8
        qk_cols = []
        for h in range(hs, hs + 8):     # interleaved [q_h, k_h] pairs
            for part in range(2):
                qk_cols.append(wq4[:, h * 3 * DH + part * DH:
                                   h * 3 * DH + (part + 1) * DH])
        v_cols = [wq4[:, h * 3 * DH + 2 * DH: h * 3 * DH + 3 * DH]
                  for h in range(hs, hs + 8)]
        kv = np.concatenate(v_cols, 1)             # (E, 1024)
        # wv4c layout [vb, p, e, n]: element = kv[128e + p, vb*512 + n]
        wv4c = np.ascontiguousarray(
            kv.reshape(NE, P, 2, TL).transpose(2, 1, 0, 3)
            .reshape(2 * P, NE * TL))
        # chunk-order permutation for the 4-way ReduceScatter pipeline
        swi_perm = [m for j in range(4)
                    for m in (2 * j, 2 * j + 1, 8 + 2 * j, 9 + 2 * j)]
        by_par[par] = {
            'wqk4c': pk(np.concatenate(qk_cols, 1)).astype(BF),
            'wv4c': wv4c.astype(BF),
            'wswiT4c': pk(np.ascontiguousarray(
                wswi4[hs * DH:(hs + 8) * DH, :]), perm=swi_perm).astype(BF),
            'gate': np.full((P, 1), float(par), f),
        }

    x = np.asarray(inputs['x'], f)
    in_maps = []
    for c in range(NCORES):
        b, par = c // 2, c % 2
        m = dict(shared)
        m.update(by_par[par])
        m['xT0'] = np.ascontiguousarray(x[b, par * TL:(par + 1) * TL, :].T)
        in_maps.append(m)
    return in_maps


_cached = {}


def kernel(**inputs):
    if 'nc' not in _cached:
        _cached['nc'] = build_program()
    nc = _cached['nc']
    in_maps = _host_inputs(inputs)
    trace = bool(int(os.environ.get('BASS_KERNEL_TRACE', '0')))
    res = run_bass_kernel_spmd(nc, in_maps, core_ids=list(range(NCORES)),
                               trace=trace)
    _cached['last_results'] = res
    out = np.zeros((4, T, E), np.float32)
    for c in range(NCORES):
        b, par = c // 2, c % 2
        out[b, par * TL:(par + 1) * TL, :] = res.results[c]['outT'].T
    return out
